# revision 27
# baseline (speedup 1.0000x reference)
"""Trainium2 Bass kernel for nn_BilinearHead (RMSNorm -> two 1x1 convs ->
bilinear scores at fixed index pairs + promo bias).

Math (per batch b):
    rms2[b]    = mean(x[b]**2) + eps
    f[b]       = from_w @ (x[b] * norm_weight) ;  t[b] = to_w @ (...)
    score[b,v] = <f[b,:,from_idx[v]], t[b,:,to_idx[v]]> / rms2[b]
                 + promo_bias[promo_idx[v]]
(valid because norm_weight == 1 and the conv biases are 0 for this problem's
input distribution; kernel() verifies and falls back to a host reference
otherwise).

Device algorithm (pure data parallel over batch: 8 cores x 128 batches),
all-fp16 on device (fp32 matmuls are 4x slower on TRN2 PE and double the
HBM traffic):

  1. Host pre-packs x as fp16 [cp=128, b=128, par=2, hw=64] so each group
     DMA is 4KB contiguous per partition.
  2. Per batch-group of 16: DVE squares (fp16 2x mode), GPSIMD halves,
     DVE reduce -> z[cp, b] partial sums of x^2.
  3. PE GEMM (fp16, parity-packed stacked weights): psum rows 0-63 =
     even-batch d, 64-127 = odd-batch d -> f, t; ACT-evict fp16.
  4. PE pair-packed Gt matmuls (row groups 0-63 / 64-127, separate psum
     banks) -> Gt_even/Gt_odd [64 j, 64 i] per batch; ACT-evict to
     gt[64 j, 128 b, 64 i] (contiguous inner runs for eviction speed).
  5. PE transpose z -> DVE reduce/scale/recip -> inv[b] = 1/rms2[b].
  6. PE one-hot matmuls, one per distinct from_idx value i (v sorted by
     from_idx on host): lhsT = gt[:, i, :], rhs = one-hot(to_idx) -> psum
     score with batch on partitions, columns in from_idx-sorted order.
  7. Fused finalize per psum chunk: out = score * inv[b] + promo_sorted
     (scalar_tensor_tensor) -> fp16 -> DMA out.
  8. Host un-sorts columns and casts fp32.
"""

import sys

sys.path.insert(0, "/opt/trn_rl_repo")

import numpy as np

import concourse.bass as bass
import concourse.tile as tile
from concourse import mybir
from concourse.bacc import Bacc
from concourse.bass_utils import run_bass_kernel_spmd

# Problem shape (hardcoded per contest contract)
B_TOT, C, HW, D, V = 1024, 256, 64, 64, 1968
N_CORES = 8
B = B_TOT // N_CORES  # 128 batches per core
CP = C // 2  # 128 channel pairs (partition dim for GEMM)
NGROUPS = 8
GB = B // NGROUPS  # 16 batches per group
PAIRS_PER_GROUP = GB // 2
EPS = 1e-6
F32 = mybir.dt.float32
F16 = mybir.dt.float16

# ---- engine-assignment knobs (tuned against the NTFF trace) ----
# batches per group whose squares run on ACT / GPSIMD (rest on DVE 2x fp16
# mult); balances ACT (evictions) vs DVE (squares + folds + reduce) vs
# GPSIMD (otherwise idle)
SQ_ACT_B = 0
SQ_GP_B = 6
# finalize (score*inv + promo) on gpsimd instead of DVE
# (False: GPSIMD has no PSUM access on TRN2 — BIR verifier rejects it)
STT_ON_GPSIMD = False


def build_kernel(seg_plan):
    """seg_plan: list of (i, col0, ncols) score-matmul segments, where i is
    the from_idx value, col0 the starting column in from_idx-sorted order,
    and the segment does not cross a 512 psum-bank boundary."""
    nc = Bacc()

    xs = nc.dram_tensor("xs", [CP, B, 2, HW], F16, kind="ExternalInput")
    # all four stacked conv weights in one upload: [4 = (f_lo,f_hi,t_lo,t_hi), par, cp, 128]
    wpack = nc.dram_tensor("wpack", [4, 2, CP, 128], F16, kind="ExternalInput")
    ident = nc.dram_tensor("ident", [128, 128], F32, kind="ExternalInput")
    # cols 0:V = one-hot(to) on rows 0-63; cols V:2V = promo broadcast
    combo = nc.dram_tensor("combo", [128, 2 * V], F16, kind="ExternalInput")
    out = nc.dram_tensor("out", [B, V], F16, kind="ExternalOutput")

    with tile.TileContext(nc) as tc:
        with (
            tc.tile_pool(name="const", bufs=1) as const,
            tc.tile_pool(name="xin", bufs=3) as xin,
            tc.tile_pool(name="x2p", bufs=3) as x2p,
            tc.tile_pool(name="x2h", bufs=2) as x2h,
            tc.tile_pool(name="ft", bufs=2) as ftp,
            tc.tile_pool(name="psmm", bufs=2, space="PSUM") as psmm,
            tc.tile_pool(name="psgt", bufs=1, space="PSUM") as psgt,
            tc.tile_pool(name="pssc", bufs=2, space="PSUM") as pssc,
        ):
            # ---- constants ----
            # x loads go on the Sync DGE queue; const loads issue in
            # parallel from the ACT DGE queue so the head is not serialized
            # on one engine's ~650ns-per-DMA setup time.
            wall = const.tile([CP, 4, 2, 128], F16)
            nc.scalar.dma_start(
                out=wall, in_=wpack[:, :, :, :].rearrange("four par cp m -> cp four par m")
            )
            ident_sb = const.tile([128, 128], F32)
            nc.scalar.dma_start(out=ident_sb, in_=ident[:, :])
            combo_sb = const.tile([128, 2 * V], F16)
            nc.scalar.dma_start(out=combo_sb, in_=combo[:, :])
            # ---- persistent working tiles ----
            gt_sb = const.tile([D, B, D], F16)  # [j, b, i]
            z = const.tile([128, B], F32)  # [cp, b] partial x^2 sums
            final_sb = const.tile([128, V], F16)
            inv_sb = const.tile([128, 1], F32)

            # score psum chunks (column-partitioned, live across the fi loop)
            n_chunks = (V + 511) // 512
            sc_ps = []
            for _q in range(n_chunks):
                sc_chunk = pssc.tile([128, 512], F32, tag="sc")
                sc_ps.append(sc_chunk)

            # ---- main loop over batch groups ----
            # x DMAs issue two groups ahead (xin bufs=3).
            def issue_x(g):
                xt = xin.tile([CP, GB, 2, HW], F16)
                nc.sync.dma_start(out=xt, in_=xs[:, g * GB : (g + 1) * GB, :, :])
                return xt

            xts = {0: issue_x(0), 1: issue_x(1)}

            for g in range(NGROUPS):
                b0 = g * GB
                xt = xts[g]
                if g + 2 < NGROUPS:
                    xts[g + 2] = issue_x(g + 2)

                # x^2 per-batch sums. Squares split ACT/DVE; then two DVE
                # fold-adds (2x fp16 mode) and one DVE reduce. All DVE work
                # is same-engine so the chain can't serialize across
                # engines (GPSIMD's tensor_add is ~4x slower than DVE's —
                # keep it out of this path entirely).
                x2t = x2p.tile([128, GB, 2 * HW], F16)
                xflat = xt[:, :, :, :].rearrange("p b par hw -> p b (par hw)")
                b1 = SQ_ACT_B
                b2 = SQ_ACT_B + SQ_GP_B
                if SQ_ACT_B > 0:
                    nc.scalar.activation(
                        out=x2t[:, 0:b1, :],
                        in_=xflat[:, 0:b1, :],
                        func=mybir.ActivationFunctionType.Square,
                    )
                if SQ_GP_B > 0:
                    nc.gpsimd.tensor_mul(
                        out=x2t[:, b1:b2, :],
                        in0=xflat[:, b1:b2, :],
                        in1=xflat[:, b1:b2, :],
                    )
                nc.vector.tensor_mul(
                    out=x2t[:, b2:GB, :],
                    in0=xflat[:, b2:GB, :],
                    in1=xflat[:, b2:GB, :],
                )
                xh1 = x2h.tile([128, GB, HW], F16, tag="h1")
                nc.vector.tensor_add(
                    out=xh1[:, :, :],
                    in0=x2t[:, :, 0:HW],
                    in1=x2t[:, :, HW : 2 * HW],
                )
                xh2 = x2h.tile([128, GB, HW // 2], F16, tag="h2")
                nc.vector.tensor_add(
                    out=xh2[:, :, :],
                    in0=xh1[:, :, 0 : HW // 2],
                    in1=xh1[:, :, HW // 2 : HW],
                )
                nc.vector.tensor_reduce(
                    out=z[:, b0 : b0 + GB],
                    in_=xh2[:, :, :],
                    axis=mybir.AxisListType.X,
                    op=mybir.AluOpType.add,
                )

                # GEMMs: psum rows 0-63 = even-batch d, rows 64-127 = odd-batch d
                # f and t go to adjacent psum banks of one tile so the
                # eviction is a single full-lane ACT copy.
                xv = xt[:, :, :, :].rearrange("p (pr two) par hw -> p pr two par hw", two=2)
                ps2 = psmm.tile([128, 2, PAIRS_PER_GROUP, HW], F32, tag="ps2")
                for fi, w0 in ((0, 0), (1, 2)):
                    for mi in range(4):
                        half, par0 = mi // 2, mi % 2
                        nc.tensor.matmul(
                            out=ps2[:, fi, :, :],
                            lhsT=wall[:, w0 + half, par0, :],
                            rhs=xv[:, :, half, par0, :],
                            start=(mi == 0),
                            stop=(mi == 3),
                        )
                ft_sb = ftp.tile([128, 2, PAIRS_PER_GROUP, HW], F16, tag="ft")
                nc.scalar.copy(out=ft_sb[:, :, :, :], in_=ps2[:, :, :, :])

                # pair-packed Gt matmuls: Gt_b[j, i] = sum_d t[d,j] f[d,i]
                # The two row groups MUST write different psum banks:
                # concurrent row-tiled PE writes to one bank kill the HW run.
                pgt2 = psgt.tile([D, 2, PAIRS_PER_GROUP, D], F32, tag="g2")
                for w in range(PAIRS_PER_GROUP):
                    nc.tensor.matmul(
                        out=pgt2[:, 0, w, :],
                        lhsT=ft_sb[0:64, 1, w, :],
                        rhs=ft_sb[0:64, 0, w, :],
                        start=True,
                        stop=True,
                        tile_position=(0, 0),
                    )
                    nc.tensor.matmul(
                        out=pgt2[:, 1, w, :],
                        lhsT=ft_sb[64:128, 1, w, :],
                        rhs=ft_sb[64:128, 0, w, :],
                        start=True,
                        stop=True,
                        tile_position=(64, 0),
                    )
                # single evict [j, (q, pair), i] -> gt[j, b, i], b = 2*(g*8+pr)+q
                # (contiguous 64-elem inner runs; strided writes are ~4x
                # slower on ACT)
                nc.scalar.copy(
                    out=gt_sb[:, b0 : b0 + GB, :].rearrange(
                        "j (pr q) i -> j q pr i", q=2
                    ),
                    in_=pgt2[:, :, :, :],
                )

            # ---- 1/rms2 per batch (natural b order on partitions) ----
            # transpose lands in score-chunk 0's psum bank (reused before
            # the score matmuls overwrite it)
            zt_ps = sc_ps[0]
            nc.tensor.transpose(out=zt_ps[:, 0:128], in_=z[:, :], identity=ident_sb[:, :])
            nc.vector.tensor_reduce(
                out=inv_sb[:, :],
                in_=zt_ps[:, 0:128],
                axis=mybir.AxisListType.X,
                op=mybir.AluOpType.add,
            )
            nc.vector.tensor_scalar(
                out=inv_sb[:, :],
                in0=inv_sb[:, :],
                scalar1=1.0 / (C * HW),
                scalar2=EPS,
                op0=mybir.AluOpType.mult,
                op1=mybir.AluOpType.add,
            )
            nc.vector.reciprocal(out=inv_sb[:, :], in_=inv_sb[:, :])

            # ---- one-hot score matmuls (columns in from_idx-sorted order) ----
            for i, col0, ncols in seg_plan:
                q, c0 = col0 // 512, col0 % 512
                nc.tensor.matmul(
                    out=sc_ps[q][:, c0 : c0 + ncols],
                    lhsT=gt_sb[:, :, i],
                    rhs=combo_sb[0:64, col0 : col0 + ncols],
                    start=True,
                    stop=True,
                )

            # ---- fused finalize: out = score * inv[b] + promo_sorted ----
            stt_eng = nc.gpsimd if STT_ON_GPSIMD else nc.vector
            for q in range(n_chunks):
                n = min(512, V - q * 512)
                stt_eng.scalar_tensor_tensor(
                    out=final_sb[:, q * 512 : q * 512 + n],
                    in0=sc_ps[q][:, 0:n],
                    scalar=inv_sb[:, 0:1],
                    in1=combo_sb[:, V + q * 512 : V + q * 512 + n],
                    op0=mybir.AluOpType.mult,
                    op1=mybir.AluOpType.add,
                )
                # per-chunk store so the DMA overlaps later chunks' finalize
                nc.sync.dma_start(
                    out=out[:, q * 512 : q * 512 + n],
                    in_=final_sb[:, q * 512 : q * 512 + n],
                )

    nc.compile()
    return nc


_NC_CACHE = {}


def _plan_from_indices(from_idx, to_idx):
    from_idx = np.asarray(from_idx, np.int64)
    to_idx = np.asarray(to_idx, np.int64)
    order = np.argsort(from_idx, kind="stable")
    fi_sorted = from_idx[order]
    seg_plan = []
    col = 0
    for i in range(HW):
        n = int(np.count_nonzero(fi_sorted == i))
        while n > 0:
            m = min(n, 512 - col % 512)
            seg_plan.append((i, col, m))
            col += m
            n -= m
    assert col == V
    onehot = np.zeros((D, V), np.float16)
    onehot[to_idx[order], np.arange(V)] = 1.0
    return tuple(seg_plan), onehot, order


def _host_inputs(from_w, to_w):
    def stack_w(wmat):
        wt = np.ascontiguousarray(wmat.T).reshape(CP, 2, D)  # [cp, par, d]
        lo = np.zeros((2, CP, 128), np.float16)
        hi = np.zeros((2, CP, 128), np.float16)
        lo[:, :, 0:D] = wt.transpose(1, 0, 2)
        hi[:, :, D:128] = wt.transpose(1, 0, 2)
        return lo, hi

    wf_lo, wf_hi = stack_w(np.asarray(from_w, np.float32))
    wt_lo, wt_hi = stack_w(np.asarray(to_w, np.float32))
    return wf_lo, wf_hi, wt_lo, wt_hi


def _device_inputs(x, from_w, to_w, promo_bias, from_idx, to_idx, promo_idx):
    """Build (seg_plan, shared input map, per-core xs list, unsort order)."""
    seg_plan, onehot, order = _plan_from_indices(from_idx, to_idx)
    wf_lo, wf_hi, wt_lo, wt_hi = _host_inputs(from_w, to_w)
    wpack = np.stack([wf_lo, wf_hi, wt_lo, wt_hi], axis=0)  # [4, 2, CP, 128]
    promo = np.asarray(promo_bias, np.float32)[np.asarray(promo_idx, np.int64)]
    combo = np.zeros((128, 2 * V), np.float16)
    combo[0:D, 0:V] = onehot
    combo[:, V : 2 * V] = promo[order].astype(np.float16)[None, :]
    shared = {
        "wpack": wpack,
        "ident": np.eye(128, dtype=np.float32),
        "combo": combo,
    }
    # x [B_TOT, C, HW] -> per-core [cp, b, par, hw] fp16 (4KB contiguous
    # per partition per group DMA)
    xr = np.asarray(x, np.float32).reshape(B_TOT, C, HW)
    xs_list = []
    for c in range(N_CORES):
        xc = xr[c * B : (c + 1) * B].reshape(B, CP, 2, HW)
        xs_list.append(np.ascontiguousarray(xc.transpose(1, 0, 2, 3)).astype(np.float16))
    return seg_plan, shared, xs_list, order


def kernel(
    x,
    norm_weight,
    from_w,
    from_b,
    to_w,
    to_b,
    promo_bias,
    from_idx,
    to_idx,
    promo_idx,
):
    x = np.asarray(x, np.float32)
    norm_weight = np.asarray(norm_weight, np.float32)
    from_b = np.asarray(from_b, np.float32)
    to_b = np.asarray(to_b, np.float32)

    if (
        np.any(from_b != 0.0)
        or np.any(to_b != 0.0)
        or not np.allclose(norm_weight, 1.0)
    ):
        # General-correctness fallback; never hit for this problem's input
        # distribution (norm_weight is ones, conv biases are zeros).
        return _host_reference(
            x, norm_weight, from_w, from_b, to_w, to_b, promo_bias,
            from_idx, to_idx, promo_idx,
        )

    seg_plan, shared, xs_list, order = _device_inputs(
        x, from_w, to_w, promo_bias, from_idx, to_idx, promo_idx
    )
    if seg_plan not in _NC_CACHE:
        _NC_CACHE[seg_plan] = build_kernel(seg_plan)
    nc = _NC_CACHE[seg_plan]

    in_maps = [dict(shared, xs=xs_list[c]) for c in range(N_CORES)]
    res = run_bass_kernel_spmd(nc, in_maps, core_ids=list(range(N_CORES)))
    full = np.empty((B_TOT, V), np.float32)
    for c in range(N_CORES):
        dev = np.asarray(res.results[c]["out"], np.float32)  # sorted columns
        full[c * B : (c + 1) * B, order] = dev
    return full


def _host_reference(
    x, norm_weight, from_w, from_b, to_w, to_b, promo_bias, from_idx, to_idx, promo_idx
):
    b, c, w, h = x.shape
    rms = np.sqrt(np.mean(x * x, axis=(1, 2, 3), keepdims=True) + EPS)
    xn = (x / rms) * norm_weight[None]
    f = (
        np.einsum("bchw,dc->bdhw", xn, from_w) + from_b[None, :, None, None]
    ).reshape(b, -1, w * h)
    t = (
        np.einsum("bchw,dc->bdhw", xn, to_w) + to_b[None, :, None, None]
    ).reshape(b, -1, w * h)
    score = np.einsum("bdv,bdv->bv", f[:, :, from_idx], t[:, :, to_idx])
    return (score + promo_bias[promo_idx][None, :]).astype(np.float32)


# revision 30
# speedup vs baseline: 1.1417x; 1.1417x over previous
"""Trainium2 Bass kernel for nn_BilinearHead (RMSNorm -> two 1x1 convs ->
bilinear scores at fixed index pairs + promo bias).

Math (per batch b):
    rms2[b]    = mean(x[b]**2) + eps
    f[b]       = from_w @ (x[b] * norm_weight) ;  t[b] = to_w @ (...)
    score[b,v] = <f[b,:,from_idx[v]], t[b,:,to_idx[v]]> / rms2[b]
                 + promo_bias[promo_idx[v]]
(valid because norm_weight == 1 and the conv biases are 0 for this problem's
input distribution; kernel() verifies and falls back to a host reference
otherwise).

Device algorithm (pure data parallel over batch: 8 cores x 128 batches),
all-fp16 on device (fp32 matmuls are 4x slower on TRN2 PE and double the
HBM traffic):

  1. Host pre-packs x as fp16 [cp=128, b=128, par=2, hw=64] so each group
     DMA is 4KB contiguous per partition.
  2. Per batch-group of 16: DVE squares (fp16 2x mode), GPSIMD halves,
     DVE reduce -> z[cp, b] partial sums of x^2.
  3. PE GEMM (fp16, parity-packed stacked weights): psum rows 0-63 =
     even-batch d, 64-127 = odd-batch d -> f, t; ACT-evict fp16.
  4. PE pair-packed Gt matmuls (row groups 0-63 / 64-127, separate psum
     banks) -> Gt_even/Gt_odd [64 j, 64 i] per batch; ACT-evict to
     gt[64 j, 128 b, 64 i] (contiguous inner runs for eviction speed).
  5. PE transpose z -> DVE reduce/scale/recip -> inv[b] = 1/rms2[b].
  6. PE one-hot matmuls, one per distinct from_idx value i (v sorted by
     from_idx on host): lhsT = gt[:, i, :], rhs = one-hot(to_idx) -> psum
     score with batch on partitions, columns in from_idx-sorted order.
  7. Fused finalize per psum chunk: out = score * inv[b] + promo_sorted
     (scalar_tensor_tensor) -> fp16 -> DMA out.
  8. Host un-sorts columns and casts fp32.
"""

import sys

sys.path.insert(0, "/opt/trn_rl_repo")

import numpy as np

import concourse.bass as bass
import concourse.tile as tile
from concourse import mybir
from concourse.bacc import Bacc
from concourse.bass_utils import run_bass_kernel_spmd

# Problem shape (hardcoded per contest contract)
B_TOT, C, HW, D, V = 1024, 256, 64, 64, 1968
N_CORES = 8
B = B_TOT // N_CORES  # 128 batches per core
CP = C // 2  # 128 channel pairs (partition dim for GEMM)
NGROUPS = 8
GB = B // NGROUPS  # 16 batches per group
PAIRS_PER_GROUP = GB // 2
EPS = 1e-6
F32 = mybir.dt.float32
F16 = mybir.dt.float16

# ---- engine-assignment knobs (tuned against the NTFF trace) ----
# batches per group whose squares run on ACT / GPSIMD (rest on DVE 2x fp16
# mult); balances ACT (evictions) vs DVE (squares + folds + reduce) vs
# GPSIMD (otherwise idle)
SQ_ACT_B = 4
SQ_GP_B = 0
# finalize (score*inv + promo) on gpsimd instead of DVE
# (False: GPSIMD has no PSUM access on TRN2 — BIR verifier rejects it)
STT_ON_GPSIMD = False


def build_kernel(seg_plan):
    """seg_plan: list of (i, col0, ncols) score-matmul segments, where i is
    the from_idx value, col0 the starting column in from_idx-sorted order,
    and the segment does not cross a 512 psum-bank boundary."""
    nc = Bacc()

    xs = nc.dram_tensor("xs", [CP, B, 2, HW], F16, kind="ExternalInput")
    # all four stacked conv weights in one upload: [4 = (f_lo,f_hi,t_lo,t_hi), par, cp, 128]
    wpack = nc.dram_tensor("wpack", [4, 2, CP, 128], F16, kind="ExternalInput")
    ident = nc.dram_tensor("ident", [128, 128], F32, kind="ExternalInput")
    # cols 0:V = one-hot(to) on rows 0-63; cols V:2V = promo broadcast
    combo = nc.dram_tensor("combo", [128, 2 * V], F16, kind="ExternalInput")
    out = nc.dram_tensor("out", [B, V], F16, kind="ExternalOutput")

    with tile.TileContext(nc) as tc:
        with (
            tc.tile_pool(name="const", bufs=1) as const,
            tc.tile_pool(name="xin", bufs=3) as xin,
            tc.tile_pool(name="x2p", bufs=3) as x2p,
            tc.tile_pool(name="x2h", bufs=2) as x2h,
            tc.tile_pool(name="ft", bufs=2) as ftp,
            tc.tile_pool(name="psmm", bufs=2, space="PSUM") as psmm,
            tc.tile_pool(name="psgt", bufs=1, space="PSUM") as psgt,
            tc.tile_pool(name="pssc", bufs=2, space="PSUM") as pssc,
        ):
            # ---- constants ----
            # x loads go on the Sync DGE queue; const loads issue in
            # parallel from the ACT DGE queue so the head is not serialized
            # on one engine's ~650ns-per-DMA setup time.
            wall = const.tile([CP, 4, 2, 128], F16)
            nc.scalar.dma_start(
                out=wall, in_=wpack[:, :, :, :].rearrange("four par cp m -> cp four par m")
            )
            ident_sb = const.tile([128, 128], F32)
            nc.scalar.dma_start(out=ident_sb, in_=ident[:, :])
            combo_sb = const.tile([128, 2 * V], F16)
            nc.scalar.dma_start(out=combo_sb, in_=combo[:, :])
            # ---- persistent working tiles ----
            gt_sb = const.tile([D, B, D], F16)  # [j, b, i]
            z = const.tile([128, B], F32)  # [cp, b] partial x^2 sums
            final_sb = const.tile([128, V], F16)
            inv_sb = const.tile([128, 1], F32)

            # score psum chunks (column-partitioned, live across the fi loop)
            n_chunks = (V + 511) // 512
            sc_ps = []
            for _q in range(n_chunks):
                sc_chunk = pssc.tile([128, 512], F32, tag="sc")
                sc_ps.append(sc_chunk)

            # ---- main loop over batch groups ----
            # x DMAs issue two groups ahead (xin bufs=3).
            def issue_x(g):
                xt = xin.tile([CP, GB, 2, HW], F16)
                nc.sync.dma_start(out=xt, in_=xs[:, g * GB : (g + 1) * GB, :, :])
                return xt

            xts = {0: issue_x(0), 1: issue_x(1)}

            # PE warm-up burst while waiting for group 0's x: the HAM clock
            # gate needs ~3.4us of sustained matmul activity to lift the PE
            # from 1.2 to 2.4 GHz; junk matmuls on the already-landed weight
            # tile start that clock during the otherwise-idle head.
            wu_ps = psgt.tile([D, 2, PAIRS_PER_GROUP, D], F32, tag="g2")
            for k in range(36):
                nc.tensor.matmul(
                    out=wu_ps[:, 0, k % PAIRS_PER_GROUP, :],
                    lhsT=wall[:, 0, 0, 0:64],
                    rhs=wall[:, 1, 0, 0:64],
                    start=True,
                    stop=True,
                    tile_position=(0, 0),
                )

            for g in range(NGROUPS):
                b0 = g * GB
                xt = xts[g]
                if g + 2 < NGROUPS:
                    xts[g + 2] = issue_x(g + 2)

                # x^2 per-batch sums. Squares split ACT/DVE; then two DVE
                # fold-adds (2x fp16 mode) and one DVE reduce. All DVE work
                # is same-engine so the chain can't serialize across
                # engines (GPSIMD's tensor_add is ~4x slower than DVE's —
                # keep it out of this path entirely).
                x2t = x2p.tile([128, GB, 2 * HW], F16)
                xflat = xt[:, :, :, :].rearrange("p b par hw -> p b (par hw)")
                b1 = SQ_ACT_B
                b2 = SQ_ACT_B + SQ_GP_B
                if SQ_ACT_B > 0:
                    nc.scalar.activation(
                        out=x2t[:, 0:b1, :],
                        in_=xflat[:, 0:b1, :],
                        func=mybir.ActivationFunctionType.Square,
                    )
                if SQ_GP_B > 0:
                    nc.gpsimd.tensor_mul(
                        out=x2t[:, b1:b2, :],
                        in0=xflat[:, b1:b2, :],
                        in1=xflat[:, b1:b2, :],
                    )
                nc.vector.tensor_mul(
                    out=x2t[:, b2:GB, :],
                    in0=xflat[:, b2:GB, :],
                    in1=xflat[:, b2:GB, :],
                )
                xh1 = x2h.tile([128, GB, HW], F16, tag="h1")
                nc.vector.tensor_add(
                    out=xh1[:, :, :],
                    in0=x2t[:, :, 0:HW],
                    in1=x2t[:, :, HW : 2 * HW],
                )
                xh2 = x2h.tile([128, GB, HW // 2], F16, tag="h2")
                nc.vector.tensor_add(
                    out=xh2[:, :, :],
                    in0=xh1[:, :, 0 : HW // 2],
                    in1=xh1[:, :, HW // 2 : HW],
                )
                nc.vector.tensor_reduce(
                    out=z[:, b0 : b0 + GB],
                    in_=xh2[:, :, :],
                    axis=mybir.AxisListType.X,
                    op=mybir.AluOpType.add,
                )

                # GEMMs: psum rows 0-63 = even-batch d, rows 64-127 = odd-batch d
                # f and t go to adjacent psum banks of one tile so the
                # eviction is a single full-lane ACT copy.
                xv = xt[:, :, :, :].rearrange("p (pr two) par hw -> p pr two par hw", two=2)
                ps2 = psmm.tile([128, 2, PAIRS_PER_GROUP, HW], F32, tag="ps2")
                for fi, w0 in ((0, 0), (1, 2)):
                    for mi in range(4):
                        half, par0 = mi // 2, mi % 2
                        nc.tensor.matmul(
                            out=ps2[:, fi, :, :],
                            lhsT=wall[:, w0 + half, par0, :],
                            rhs=xv[:, :, half, par0, :],
                            start=(mi == 0),
                            stop=(mi == 3),
                        )
                ft_sb = ftp.tile([128, 2, PAIRS_PER_GROUP, HW], F16, tag="ft")
                nc.scalar.copy(out=ft_sb[:, :, :, :], in_=ps2[:, :, :, :])

                # pair-packed Gt matmuls: Gt_b[j, i] = sum_d t[d,j] f[d,i]
                # The two row groups MUST write different psum banks:
                # concurrent row-tiled PE writes to one bank kill the HW run.
                pgt2 = psgt.tile([D, 2, PAIRS_PER_GROUP, D], F32, tag="g2")
                for w in range(PAIRS_PER_GROUP):
                    nc.tensor.matmul(
                        out=pgt2[:, 0, w, :],
                        lhsT=ft_sb[0:64, 1, w, :],
                        rhs=ft_sb[0:64, 0, w, :],
                        start=True,
                        stop=True,
                        tile_position=(0, 0),
                    )
                    nc.tensor.matmul(
                        out=pgt2[:, 1, w, :],
                        lhsT=ft_sb[64:128, 1, w, :],
                        rhs=ft_sb[64:128, 0, w, :],
                        start=True,
                        stop=True,
                        tile_position=(64, 0),
                    )
                # single evict [j, (q, pair), i] -> gt[j, b, i], b = 2*(g*8+pr)+q
                # (contiguous 64-elem inner runs; strided writes are ~4x
                # slower on ACT)
                nc.scalar.copy(
                    out=gt_sb[:, b0 : b0 + GB, :].rearrange(
                        "j (pr q) i -> j q pr i", q=2
                    ),
                    in_=pgt2[:, :, :, :],
                )

            # ---- 1/rms2 per batch (natural b order on partitions) ----
            # transpose lands in score-chunk 0's psum bank (reused before
            # the score matmuls overwrite it)
            zt_ps = sc_ps[0]
            nc.tensor.transpose(out=zt_ps[:, 0:128], in_=z[:, :], identity=ident_sb[:, :])
            nc.vector.tensor_reduce(
                out=inv_sb[:, :],
                in_=zt_ps[:, 0:128],
                axis=mybir.AxisListType.X,
                op=mybir.AluOpType.add,
            )
            nc.vector.tensor_scalar(
                out=inv_sb[:, :],
                in0=inv_sb[:, :],
                scalar1=1.0 / (C * HW),
                scalar2=EPS,
                op0=mybir.AluOpType.mult,
                op1=mybir.AluOpType.add,
            )
            nc.vector.reciprocal(out=inv_sb[:, :], in_=inv_sb[:, :])

            # ---- one-hot score matmuls (columns in from_idx-sorted order) ----
            for i, col0, ncols in seg_plan:
                q, c0 = col0 // 512, col0 % 512
                nc.tensor.matmul(
                    out=sc_ps[q][:, c0 : c0 + ncols],
                    lhsT=gt_sb[:, :, i],
                    rhs=combo_sb[0:64, col0 : col0 + ncols],
                    start=True,
                    stop=True,
                )

            # ---- fused finalize: out = score * inv[b] + promo_sorted ----
            stt_eng = nc.gpsimd if STT_ON_GPSIMD else nc.vector
            for q in range(n_chunks):
                n = min(512, V - q * 512)
                stt_eng.scalar_tensor_tensor(
                    out=final_sb[:, q * 512 : q * 512 + n],
                    in0=sc_ps[q][:, 0:n],
                    scalar=inv_sb[:, 0:1],
                    in1=combo_sb[:, V + q * 512 : V + q * 512 + n],
                    op0=mybir.AluOpType.mult,
                    op1=mybir.AluOpType.add,
                )
                # per-chunk store so the DMA overlaps later chunks' finalize;
                # alternate DGE queues so the ~600ns issue times overlap too
                dma_eng = nc.sync if q % 2 == 0 else nc.scalar
                dma_eng.dma_start(
                    out=out[:, q * 512 : q * 512 + n],
                    in_=final_sb[:, q * 512 : q * 512 + n],
                )

    nc.compile()
    return nc


_NC_CACHE = {}


def _plan_from_indices(from_idx, to_idx):
    from_idx = np.asarray(from_idx, np.int64)
    to_idx = np.asarray(to_idx, np.int64)
    order = np.argsort(from_idx, kind="stable")
    fi_sorted = from_idx[order]
    seg_plan = []
    col = 0
    for i in range(HW):
        n = int(np.count_nonzero(fi_sorted == i))
        while n > 0:
            m = min(n, 512 - col % 512)
            seg_plan.append((i, col, m))
            col += m
            n -= m
    assert col == V
    onehot = np.zeros((D, V), np.float16)
    onehot[to_idx[order], np.arange(V)] = 1.0
    return tuple(seg_plan), onehot, order


def _host_inputs(from_w, to_w):
    def stack_w(wmat):
        wt = np.ascontiguousarray(wmat.T).reshape(CP, 2, D)  # [cp, par, d]
        lo = np.zeros((2, CP, 128), np.float16)
        hi = np.zeros((2, CP, 128), np.float16)
        lo[:, :, 0:D] = wt.transpose(1, 0, 2)
        hi[:, :, D:128] = wt.transpose(1, 0, 2)
        return lo, hi

    wf_lo, wf_hi = stack_w(np.asarray(from_w, np.float32))
    wt_lo, wt_hi = stack_w(np.asarray(to_w, np.float32))
    return wf_lo, wf_hi, wt_lo, wt_hi


def _device_inputs(x, from_w, to_w, promo_bias, from_idx, to_idx, promo_idx):
    """Build (seg_plan, shared input map, per-core xs list, unsort order)."""
    seg_plan, onehot, order = _plan_from_indices(from_idx, to_idx)
    wf_lo, wf_hi, wt_lo, wt_hi = _host_inputs(from_w, to_w)
    wpack = np.stack([wf_lo, wf_hi, wt_lo, wt_hi], axis=0)  # [4, 2, CP, 128]
    promo = np.asarray(promo_bias, np.float32)[np.asarray(promo_idx, np.int64)]
    combo = np.zeros((128, 2 * V), np.float16)
    combo[0:D, 0:V] = onehot
    combo[:, V : 2 * V] = promo[order].astype(np.float16)[None, :]
    shared = {
        "wpack": wpack,
        "ident": np.eye(128, dtype=np.float32),
        "combo": combo,
    }
    # x [B_TOT, C, HW] -> per-core [cp, b, par, hw] fp16 (4KB contiguous
    # per partition per group DMA)
    xr = np.asarray(x, np.float32).reshape(B_TOT, C, HW)
    xs_list = []
    for c in range(N_CORES):
        xc = xr[c * B : (c + 1) * B].reshape(B, CP, 2, HW)
        xs_list.append(np.ascontiguousarray(xc.transpose(1, 0, 2, 3)).astype(np.float16))
    return seg_plan, shared, xs_list, order


def kernel(
    x,
    norm_weight,
    from_w,
    from_b,
    to_w,
    to_b,
    promo_bias,
    from_idx,
    to_idx,
    promo_idx,
):
    x = np.asarray(x, np.float32)
    norm_weight = np.asarray(norm_weight, np.float32)
    from_b = np.asarray(from_b, np.float32)
    to_b = np.asarray(to_b, np.float32)

    if (
        np.any(from_b != 0.0)
        or np.any(to_b != 0.0)
        or not np.allclose(norm_weight, 1.0)
    ):
        # General-correctness fallback; never hit for this problem's input
        # distribution (norm_weight is ones, conv biases are zeros).
        return _host_reference(
            x, norm_weight, from_w, from_b, to_w, to_b, promo_bias,
            from_idx, to_idx, promo_idx,
        )

    seg_plan, shared, xs_list, order = _device_inputs(
        x, from_w, to_w, promo_bias, from_idx, to_idx, promo_idx
    )
    if seg_plan not in _NC_CACHE:
        _NC_CACHE[seg_plan] = build_kernel(seg_plan)
    nc = _NC_CACHE[seg_plan]

    in_maps = [dict(shared, xs=xs_list[c]) for c in range(N_CORES)]
    res = run_bass_kernel_spmd(nc, in_maps, core_ids=list(range(N_CORES)))
    full = np.empty((B_TOT, V), np.float32)
    for c in range(N_CORES):
        dev = np.asarray(res.results[c]["out"], np.float32)  # sorted columns
        full[c * B : (c + 1) * B, order] = dev
    return full


def _host_reference(
    x, norm_weight, from_w, from_b, to_w, to_b, promo_bias, from_idx, to_idx, promo_idx
):
    b, c, w, h = x.shape
    rms = np.sqrt(np.mean(x * x, axis=(1, 2, 3), keepdims=True) + EPS)
    xn = (x / rms) * norm_weight[None]
    f = (
        np.einsum("bchw,dc->bdhw", xn, from_w) + from_b[None, :, None, None]
    ).reshape(b, -1, w * h)
    t = (
        np.einsum("bchw,dc->bdhw", xn, to_w) + to_b[None, :, None, None]
    ).reshape(b, -1, w * h)
    score = np.einsum("bdv,bdv->bv", f[:, :, from_idx], t[:, :, to_idx])
    return (score + promo_bias[promo_idx][None, :]).astype(np.float32)


# revision 31
# speedup vs baseline: 1.1935x; 1.0454x over previous
"""Trainium2 Bass kernel for nn_BilinearHead (RMSNorm -> two 1x1 convs ->
bilinear scores at fixed index pairs + promo bias).

Math (per batch b):
    rms2[b]    = mean(x[b]**2) + eps
    f[b]       = from_w @ (x[b] * norm_weight) ;  t[b] = to_w @ (...)
    score[b,v] = <f[b,:,from_idx[v]], t[b,:,to_idx[v]]> / rms2[b]
                 + promo_bias[promo_idx[v]]
(valid because norm_weight == 1 and the conv biases are 0 for this problem's
input distribution; kernel() verifies and falls back to a host reference
otherwise).

Device algorithm (pure data parallel over batch: 8 cores x 128 batches),
all-fp16 on device (fp32 matmuls are 4x slower on TRN2 PE and double the
HBM traffic):

  1. Host pre-packs x as fp16 [cp=128, b=128, par=2, hw=64] so each group
     DMA is 4KB contiguous per partition.
  2. Per batch-group of 16: DVE squares (fp16 2x mode), GPSIMD halves,
     DVE reduce -> z[cp, b] partial sums of x^2.
  3. PE GEMM (fp16, parity-packed stacked weights): psum rows 0-63 =
     even-batch d, 64-127 = odd-batch d -> f, t; ACT-evict fp16.
  4. PE pair-packed Gt matmuls (row groups 0-63 / 64-127, separate psum
     banks) -> Gt_even/Gt_odd [64 j, 64 i] per batch; ACT-evict to
     gt[64 j, 128 b, 64 i] (contiguous inner runs for eviction speed).
  5. PE transpose z -> DVE reduce/scale/recip -> inv[b] = 1/rms2[b].
  6. PE one-hot matmuls, one per distinct from_idx value i (v sorted by
     from_idx on host): lhsT = gt[:, i, :], rhs = one-hot(to_idx) -> psum
     score with batch on partitions, columns in from_idx-sorted order.
  7. Fused finalize per psum chunk: out = score * inv[b] + promo_sorted
     (scalar_tensor_tensor) -> fp16 -> DMA out.
  8. Host un-sorts columns and casts fp32.
"""

import sys

sys.path.insert(0, "/opt/trn_rl_repo")

import numpy as np

import concourse.bass as bass
import concourse.tile as tile
from concourse import mybir
from concourse.bacc import Bacc
from concourse.bass_utils import run_bass_kernel_spmd

# Problem shape (hardcoded per contest contract)
B_TOT, C, HW, D, V = 1024, 256, 64, 64, 1968
N_CORES = 8
B = B_TOT // N_CORES  # 128 batches per core
CP = C // 2  # 128 channel pairs (partition dim for GEMM)
NGROUPS = 8
GB = B // NGROUPS  # 16 batches per group
PAIRS_PER_GROUP = GB // 2
EPS = 1e-6
F32 = mybir.dt.float32
F16 = mybir.dt.float16

# ---- engine-assignment knobs (tuned against the NTFF trace) ----
# batches per group whose squares run on ACT / GPSIMD (rest on DVE 2x fp16
# mult); balances ACT (evictions) vs DVE (squares + folds + reduce) vs
# GPSIMD (otherwise idle)
SQ_ACT_B = 4
SQ_GP_B = 0
# finalize (score*inv + promo) on gpsimd instead of DVE
# (False: GPSIMD has no PSUM access on TRN2 — BIR verifier rejects it)
STT_ON_GPSIMD = False


def build_kernel(seg_plan):
    """seg_plan: list of (i, col0, ncols) score-matmul segments, where i is
    the from_idx value, col0 the starting column in from_idx-sorted order,
    and the segment does not cross a 512 psum-bank boundary."""
    nc = Bacc()

    xs = nc.dram_tensor("xs", [CP, B, 2, HW], F16, kind="ExternalInput")
    # all four stacked conv weights in one upload: [4 = (f_lo,f_hi,t_lo,t_hi), par, cp, 128]
    wpack = nc.dram_tensor("wpack", [4, 2, CP, 128], F16, kind="ExternalInput")
    ident = nc.dram_tensor("ident", [128, 128], F32, kind="ExternalInput")
    # cols 0:V = one-hot(to) on rows 0-63; cols V:2V = promo broadcast
    combo = nc.dram_tensor("combo", [128, 2 * V], F16, kind="ExternalInput")
    out = nc.dram_tensor("out", [B, V], F16, kind="ExternalOutput")

    with tile.TileContext(nc) as tc:
        with (
            tc.tile_pool(name="const", bufs=1) as const,
            tc.tile_pool(name="xin", bufs=3) as xin,
            tc.tile_pool(name="x2p", bufs=3) as x2p,
            tc.tile_pool(name="x2h", bufs=2) as x2h,
            tc.tile_pool(name="ft", bufs=2) as ftp,
            tc.tile_pool(name="psmm", bufs=2, space="PSUM") as psmm,
            tc.tile_pool(name="psgt", bufs=1, space="PSUM") as psgt,
            tc.tile_pool(name="pssc", bufs=2, space="PSUM") as pssc,
        ):
            # ---- constants ----
            # x loads go on the Sync DGE queue; const loads issue in
            # parallel from the ACT DGE queue so the head is not serialized
            # on one engine's ~650ns-per-DMA setup time.
            wall = const.tile([CP, 4, 2, 128], F16)
            nc.scalar.dma_start(
                out=wall, in_=wpack[:, :, :, :].rearrange("four par cp m -> cp four par m")
            )
            ident_sb = const.tile([128, 128], F32)
            nc.scalar.dma_start(out=ident_sb, in_=ident[:, :])
            combo_sb = const.tile([128, 2 * V], F16)
            nc.scalar.dma_start(out=combo_sb, in_=combo[:, :])
            # ---- persistent working tiles ----
            gt_sb = const.tile([D, B, D], F16)  # [j, b, i]
            z = const.tile([128, B], F32)  # [cp, b] partial x^2 sums
            final_sb = const.tile([128, V], F16)
            inv_sb = const.tile([128, 1], F32)

            # score psum chunks (column-partitioned, live across the fi loop)
            n_chunks = (V + 511) // 512
            sc_ps = []
            for _q in range(n_chunks):
                sc_chunk = pssc.tile([128, 512], F32, tag="sc")
                sc_ps.append(sc_chunk)

            # ---- main loop over batch groups ----
            # x DMAs issue two groups ahead (xin bufs=3).
            def issue_x(g):
                xt = xin.tile([CP, GB, 2, HW], F16)
                nc.sync.dma_start(out=xt, in_=xs[:, g * GB : (g + 1) * GB, :, :])
                return xt

            xts = {0: issue_x(0), 1: issue_x(1)}

            # PE warm-up burst while waiting for group 0's x: the HAM clock
            # gate needs ~3.4us of sustained matmul activity to lift the PE
            # from 1.2 to 2.4 GHz. A memset tile (no DMA dependency) lets
            # the burst start right after the preamble, so group 0's GEMM
            # already runs at full clock.
            wu_w = const.tile([128, 128], F16)
            nc.vector.memset(wu_w, 0.25)
            wu_ps = psgt.tile([D, 2, PAIRS_PER_GROUP, D], F32, tag="g2")
            for k in range(52):
                nc.tensor.matmul(
                    out=wu_ps[:, 0, k % PAIRS_PER_GROUP, :],
                    lhsT=wu_w[:, 0:64],
                    rhs=wu_w[:, 64:128],
                    start=True,
                    stop=True,
                    tile_position=(0, 0),
                )

            for g in range(NGROUPS):
                b0 = g * GB
                xt = xts[g]
                if g + 2 < NGROUPS:
                    xts[g + 2] = issue_x(g + 2)

                # x^2 per-batch sums. Squares split ACT/DVE; then two DVE
                # fold-adds (2x fp16 mode) and one DVE reduce. All DVE work
                # is same-engine so the chain can't serialize across
                # engines (GPSIMD's tensor_add is ~4x slower than DVE's —
                # keep it out of this path entirely).
                x2t = x2p.tile([128, GB, 2 * HW], F16)
                xflat = xt[:, :, :, :].rearrange("p b par hw -> p b (par hw)")
                b1 = SQ_ACT_B
                b2 = SQ_ACT_B + SQ_GP_B
                if SQ_ACT_B > 0:
                    nc.scalar.activation(
                        out=x2t[:, 0:b1, :],
                        in_=xflat[:, 0:b1, :],
                        func=mybir.ActivationFunctionType.Square,
                    )
                if SQ_GP_B > 0:
                    nc.gpsimd.tensor_mul(
                        out=x2t[:, b1:b2, :],
                        in0=xflat[:, b1:b2, :],
                        in1=xflat[:, b1:b2, :],
                    )
                nc.vector.tensor_mul(
                    out=x2t[:, b2:GB, :],
                    in0=xflat[:, b2:GB, :],
                    in1=xflat[:, b2:GB, :],
                )
                xh1 = x2h.tile([128, GB, HW], F16, tag="h1")
                nc.vector.tensor_add(
                    out=xh1[:, :, :],
                    in0=x2t[:, :, 0:HW],
                    in1=x2t[:, :, HW : 2 * HW],
                )
                xh2 = x2h.tile([128, GB, HW // 2], F16, tag="h2")
                nc.vector.tensor_add(
                    out=xh2[:, :, :],
                    in0=xh1[:, :, 0 : HW // 2],
                    in1=xh1[:, :, HW // 2 : HW],
                )
                nc.vector.tensor_reduce(
                    out=z[:, b0 : b0 + GB],
                    in_=xh2[:, :, :],
                    axis=mybir.AxisListType.X,
                    op=mybir.AluOpType.add,
                )

                # GEMMs: psum rows 0-63 = even-batch d, rows 64-127 = odd-batch d
                # f and t go to adjacent psum banks of one tile so the
                # eviction is a single full-lane ACT copy.
                xv = xt[:, :, :, :].rearrange("p (pr two) par hw -> p pr two par hw", two=2)
                ps2 = psmm.tile([128, 2, PAIRS_PER_GROUP, HW], F32, tag="ps2")
                for fi, w0 in ((0, 0), (1, 2)):
                    for mi in range(4):
                        half, par0 = mi // 2, mi % 2
                        nc.tensor.matmul(
                            out=ps2[:, fi, :, :],
                            lhsT=wall[:, w0 + half, par0, :],
                            rhs=xv[:, :, half, par0, :],
                            start=(mi == 0),
                            stop=(mi == 3),
                        )
                ft_sb = ftp.tile([128, 2, PAIRS_PER_GROUP, HW], F16, tag="ft")
                nc.scalar.copy(out=ft_sb[:, :, :, :], in_=ps2[:, :, :, :])

                # pair-packed Gt matmuls: Gt_b[j, i] = sum_d t[d,j] f[d,i]
                # The two row groups MUST write different psum banks:
                # concurrent row-tiled PE writes to one bank kill the HW run.
                pgt2 = psgt.tile([D, 2, PAIRS_PER_GROUP, D], F32, tag="g2")
                for w in range(PAIRS_PER_GROUP):
                    nc.tensor.matmul(
                        out=pgt2[:, 0, w, :],
                        lhsT=ft_sb[0:64, 1, w, :],
                        rhs=ft_sb[0:64, 0, w, :],
                        start=True,
                        stop=True,
                        tile_position=(0, 0),
                    )
                    nc.tensor.matmul(
                        out=pgt2[:, 1, w, :],
                        lhsT=ft_sb[64:128, 1, w, :],
                        rhs=ft_sb[64:128, 0, w, :],
                        start=True,
                        stop=True,
                        tile_position=(64, 0),
                    )
                # single evict [j, (q, pair), i] -> gt[j, b, i], b = 2*(g*8+pr)+q
                # (contiguous 64-elem inner runs; strided writes are ~4x
                # slower on ACT)
                nc.scalar.copy(
                    out=gt_sb[:, b0 : b0 + GB, :].rearrange(
                        "j (pr q) i -> j q pr i", q=2
                    ),
                    in_=pgt2[:, :, :, :],
                )

            # ---- 1/rms2 per batch (natural b order on partitions) ----
            # transpose lands in score-chunk 0's psum bank (reused before
            # the score matmuls overwrite it)
            zt_ps = sc_ps[0]
            nc.tensor.transpose(out=zt_ps[:, 0:128], in_=z[:, :], identity=ident_sb[:, :])
            nc.vector.tensor_reduce(
                out=inv_sb[:, :],
                in_=zt_ps[:, 0:128],
                axis=mybir.AxisListType.X,
                op=mybir.AluOpType.add,
            )
            nc.vector.tensor_scalar(
                out=inv_sb[:, :],
                in0=inv_sb[:, :],
                scalar1=1.0 / (C * HW),
                scalar2=EPS,
                op0=mybir.AluOpType.mult,
                op1=mybir.AluOpType.add,
            )
            nc.vector.reciprocal(out=inv_sb[:, :], in_=inv_sb[:, :])

            # ---- one-hot score matmuls (columns in from_idx-sorted order) ----
            for i, col0, ncols in seg_plan:
                q, c0 = col0 // 512, col0 % 512
                nc.tensor.matmul(
                    out=sc_ps[q][:, c0 : c0 + ncols],
                    lhsT=gt_sb[:, :, i],
                    rhs=combo_sb[0:64, col0 : col0 + ncols],
                    start=True,
                    stop=True,
                )

            # ---- fused finalize: out = score * inv[b] + promo_sorted ----
            stt_eng = nc.gpsimd if STT_ON_GPSIMD else nc.vector
            for q in range(n_chunks):
                n = min(512, V - q * 512)
                stt_eng.scalar_tensor_tensor(
                    out=final_sb[:, q * 512 : q * 512 + n],
                    in0=sc_ps[q][:, 0:n],
                    scalar=inv_sb[:, 0:1],
                    in1=combo_sb[:, V + q * 512 : V + q * 512 + n],
                    op0=mybir.AluOpType.mult,
                    op1=mybir.AluOpType.add,
                )
                # per-chunk store so the DMA overlaps later chunks' finalize;
                # alternate DGE queues so the ~600ns issue times overlap too
                dma_eng = nc.sync if q % 2 == 0 else nc.scalar
                dma_eng.dma_start(
                    out=out[:, q * 512 : q * 512 + n],
                    in_=final_sb[:, q * 512 : q * 512 + n],
                )

    nc.compile()
    return nc


_NC_CACHE = {}


def _plan_from_indices(from_idx, to_idx):
    from_idx = np.asarray(from_idx, np.int64)
    to_idx = np.asarray(to_idx, np.int64)
    order = np.argsort(from_idx, kind="stable")
    fi_sorted = from_idx[order]
    seg_plan = []
    col = 0
    for i in range(HW):
        n = int(np.count_nonzero(fi_sorted == i))
        while n > 0:
            m = min(n, 512 - col % 512)
            seg_plan.append((i, col, m))
            col += m
            n -= m
    assert col == V
    onehot = np.zeros((D, V), np.float16)
    onehot[to_idx[order], np.arange(V)] = 1.0
    return tuple(seg_plan), onehot, order


def _host_inputs(from_w, to_w):
    def stack_w(wmat):
        wt = np.ascontiguousarray(wmat.T).reshape(CP, 2, D)  # [cp, par, d]
        lo = np.zeros((2, CP, 128), np.float16)
        hi = np.zeros((2, CP, 128), np.float16)
        lo[:, :, 0:D] = wt.transpose(1, 0, 2)
        hi[:, :, D:128] = wt.transpose(1, 0, 2)
        return lo, hi

    wf_lo, wf_hi = stack_w(np.asarray(from_w, np.float32))
    wt_lo, wt_hi = stack_w(np.asarray(to_w, np.float32))
    return wf_lo, wf_hi, wt_lo, wt_hi


def _device_inputs(x, from_w, to_w, promo_bias, from_idx, to_idx, promo_idx):
    """Build (seg_plan, shared input map, per-core xs list, unsort order)."""
    seg_plan, onehot, order = _plan_from_indices(from_idx, to_idx)
    wf_lo, wf_hi, wt_lo, wt_hi = _host_inputs(from_w, to_w)
    wpack = np.stack([wf_lo, wf_hi, wt_lo, wt_hi], axis=0)  # [4, 2, CP, 128]
    promo = np.asarray(promo_bias, np.float32)[np.asarray(promo_idx, np.int64)]
    combo = np.zeros((128, 2 * V), np.float16)
    combo[0:D, 0:V] = onehot
    combo[:, V : 2 * V] = promo[order].astype(np.float16)[None, :]
    shared = {
        "wpack": wpack,
        "ident": np.eye(128, dtype=np.float32),
        "combo": combo,
    }
    # x [B_TOT, C, HW] -> per-core [cp, b, par, hw] fp16 (4KB contiguous
    # per partition per group DMA)
    xr = np.asarray(x, np.float32).reshape(B_TOT, C, HW)
    xs_list = []
    for c in range(N_CORES):
        xc = xr[c * B : (c + 1) * B].reshape(B, CP, 2, HW)
        xs_list.append(np.ascontiguousarray(xc.transpose(1, 0, 2, 3)).astype(np.float16))
    return seg_plan, shared, xs_list, order


def kernel(
    x,
    norm_weight,
    from_w,
    from_b,
    to_w,
    to_b,
    promo_bias,
    from_idx,
    to_idx,
    promo_idx,
):
    x = np.asarray(x, np.float32)
    norm_weight = np.asarray(norm_weight, np.float32)
    from_b = np.asarray(from_b, np.float32)
    to_b = np.asarray(to_b, np.float32)

    if (
        np.any(from_b != 0.0)
        or np.any(to_b != 0.0)
        or not np.allclose(norm_weight, 1.0)
    ):
        # General-correctness fallback; never hit for this problem's input
        # distribution (norm_weight is ones, conv biases are zeros).
        return _host_reference(
            x, norm_weight, from_w, from_b, to_w, to_b, promo_bias,
            from_idx, to_idx, promo_idx,
        )

    seg_plan, shared, xs_list, order = _device_inputs(
        x, from_w, to_w, promo_bias, from_idx, to_idx, promo_idx
    )
    if seg_plan not in _NC_CACHE:
        _NC_CACHE[seg_plan] = build_kernel(seg_plan)
    nc = _NC_CACHE[seg_plan]

    in_maps = [dict(shared, xs=xs_list[c]) for c in range(N_CORES)]
    res = run_bass_kernel_spmd(nc, in_maps, core_ids=list(range(N_CORES)))
    full = np.empty((B_TOT, V), np.float32)
    for c in range(N_CORES):
        dev = np.asarray(res.results[c]["out"], np.float32)  # sorted columns
        full[c * B : (c + 1) * B, order] = dev
    return full


def _host_reference(
    x, norm_weight, from_w, from_b, to_w, to_b, promo_bias, from_idx, to_idx, promo_idx
):
    b, c, w, h = x.shape
    rms = np.sqrt(np.mean(x * x, axis=(1, 2, 3), keepdims=True) + EPS)
    xn = (x / rms) * norm_weight[None]
    f = (
        np.einsum("bchw,dc->bdhw", xn, from_w) + from_b[None, :, None, None]
    ).reshape(b, -1, w * h)
    t = (
        np.einsum("bchw,dc->bdhw", xn, to_w) + to_b[None, :, None, None]
    ).reshape(b, -1, w * h)
    score = np.einsum("bdv,bdv->bv", f[:, :, from_idx], t[:, :, to_idx])
    return (score + promo_bias[promo_idx][None, :]).astype(np.float32)
